# revision 2
# baseline (speedup 1.0000x reference)
"""Trainium2 Bass kernel for nn_DiagSSMBlock: h_t = tanh(a * h_{t-1} + (x @ b)_t).

Strategy (8 NeuronCores, 2D shard: 4 T-chunks x 2 H-halves => PE-bound):
  - The per-core GEMM work is fixed (34.4 GFLOP / 8 = 4.3 GFLOP, ~54.6us at
    the fp16 PE rate of 1 row/cycle @ 2.4 GHz), so the only lever is HBM
    traffic. The old H-only sharding broadcast the full 16MB fp16 x to every
    core (~61us DMA > PE). The 4x2 grid moves per core only:
      x slice  [K=2048, Tc=1024] fp16 = 4.2MB
      b half   [K=2048, Hc=1024] fp16 = 4.2MB
      out      [Hc, Tc]          fp16 = 2.1MB
    => ~10.5MB ~ 29us, fully hidden under the PE stream.
  - The diagonal recurrence is per-channel independent; T-sharding is made
    embarrassingly parallel by the same Gauss-Seidel fixed-point trick as
    before (|a| <= 0.03125, tanh 1-Lipschitz => each sweep contracts error
    by |a|). Chunk carries are resolved on the HOST: for each chunk start
    t0, two fp32 GEMV rows s_{t0-2}, s_{t0-1} (50 MFLOP total) give
      c1 = u0[t0-1] = tanh(s_{t0-1})
      c2 = u1[t0-1] = tanh(a*tanh(s_{t0-2}) + s_{t0-1})
    which seed sweep 1 / sweep 2 via a carry slot at U[:, 0]. Scan error
    after u0 + 2 sweeps is ~|a|^3 ~ 3e-5; the fp16 GEMM rounding (~9e-4 of
    output scale) dominates. Measured end-to-end rel err ~1.0e-3 (tol 2e-2).
  - All elementwise state (s, u, z, out) is fp16: 2x DVE rate and half the
    output DMA. Output is cast back to fp32 on host.
"""

import numpy as np

import jax
from jax.sharding import Mesh, NamedSharding, PartitionSpec
from jax.experimental.shard_map import shard_map

import concourse.tile as tile
from concourse import bacc, mybir
from concourse.bass2jax import (
    _bass_exec_p,
    install_neuronx_cc_hook,
    partition_id_tensor,
)

T = 4096          # sequence length
K = 2048          # input features (contraction dim)
H = 2048          # output channels
N_CORES = 8
P_T = 4           # t-chunks
Q_H = 2           # h-halves
Tc = T // P_T     # 1024 time steps per core
Hc = H // Q_H     # 1024 channels per core
NG = Hc // 128    # 8 channel groups of 128 partitions
KT = K // 128     # 16 k-tiles
TB = 512          # GEMM moving-dim block (one PSUM bank of fp32)
NB = Tc // TB     # 2 t-blocks

F32 = mybir.dt.float32
F16 = mybir.dt.float16


def _build(loop_iters: int):
    nc = bacc.Bacc(
        "TRN2", target_bir_lowering=False, debug=False, num_devices=N_CORES
    )

    xt_d = nc.dram_tensor("xt", [K, Tc], F16, kind="ExternalInput").ap()
    bt_d = nc.dram_tensor("bt", [K, Hc], F16, kind="ExternalInput").ap()
    cr_d = nc.dram_tensor("cr", [128, NG, 2], F16, kind="ExternalInput").ap()
    av_d = nc.dram_tensor("av", [128, NG], F16, kind="ExternalInput").ap()
    ht_d = nc.dram_tensor("ht", [Hc, Tc], F16, kind="ExternalOutput").ap()

    xt_r = xt_d.rearrange("(kt p) t -> p kt t", p=128)
    bt_r = bt_d.rearrange("(kt p) c -> p kt c", p=128)
    ht_r = ht_d.rearrange("(g p) t -> p g t", g=NG)

    Tanh = mybir.ActivationFunctionType.Tanh
    MUL = mybir.AluOpType.mult
    ADD = mybir.AluOpType.add

    with tile.TileContext(nc) as tc:
        with (
            tc.tile_pool(name="state", bufs=1) as state,
            tc.tile_pool(name="ps", bufs=6, space="PSUM") as psum,
            tc.tile_pool(name="zp", bufs=4) as zpool,
            tc.tile_pool(name="op", bufs=3) as opool,
        ):

            def body(_i):
                b_sb = state.tile([128, KT, Hc], F16, tag="b")
                x_sb = state.tile([128, KT, Tc], F16, tag="x")
                a_sb = state.tile([128, NG], F16, tag="a")
                cr_sb = state.tile([128, NG, 2], F16, tag="cr")
                U = [
                    state.tile([128, Tc + 1], F16, tag=f"U{g}", name=f"U{g}")
                    for g in range(NG)
                ]
                sT = [
                    state.tile([128, Tc], F16, tag=f"sT{g}", name=f"sT{g}")
                    for g in range(NG)
                ]

                # Input streams, 1MB chunks, ordered so the first matmuls
                # (b cols 0:256 + x block 0) are fed first. Chunking also
                # lets iteration n+1's reloads start as soon as iteration
                # n's last reader of each chunk retires (cross-iteration
                # overlap through the For_i back edge).
                nc.sync.dma_start(out=cr_sb, in_=cr_d)
                nc.sync.dma_start(out=a_sb, in_=av_d)
                nc.sync.dma_start(out=b_sb[:, :, 0:256], in_=bt_r[:, :, 0:256])
                nc.sync.dma_start(out=x_sb[:, :, 0:256], in_=xt_r[:, :, 0:256])
                nc.sync.dma_start(out=x_sb[:, :, 256:512], in_=xt_r[:, :, 256:512])
                nc.sync.dma_start(out=b_sb[:, :, 256:512], in_=bt_r[:, :, 256:512])
                nc.sync.dma_start(out=b_sb[:, :, 512:768], in_=bt_r[:, :, 512:768])
                nc.sync.dma_start(out=b_sb[:, :, 768:1024], in_=bt_r[:, :, 768:1024])
                nc.sync.dma_start(out=x_sb[:, :, 512:768], in_=xt_r[:, :, 512:768])
                nc.sync.dma_start(out=x_sb[:, :, 768:1024], in_=xt_r[:, :, 768:1024])
                for g in range(NG):
                    nc.vector.tensor_copy(
                        out=U[g][:, 0:1], in_=cr_sb[:, g, 0:1]
                    )

                def gemm(tb, g):
                    ps = psum.tile([128, TB], F32, tag="ps")
                    lo = tb * TB
                    for kt in range(KT):
                        nc.tensor.matmul(
                            ps,
                            lhsT=b_sb[:, kt, g * 128:(g + 1) * 128],
                            rhs=x_sb[:, kt, lo:lo + TB],
                            start=(kt == 0),
                            stop=(kt == KT - 1),
                        )
                    nc.vector.tensor_copy(out=sT[g][:, lo:lo + TB], in_=ps)
                    nc.scalar.activation(
                        out=U[g][:, 1 + lo:1 + lo + TB], in_=ps, func=Tanh
                    )

                def sweep(g, lo, out_ap, tag):
                    z = zpool.tile([128, TB], F16, tag="z", name=f"z_{tag}")
                    nc.vector.scalar_tensor_tensor(
                        out=z,
                        in0=U[g][:, lo:lo + TB],
                        scalar=a_sb[:, g:g + 1],
                        in1=sT[g][:, lo:lo + TB],
                        op0=MUL,
                        op1=ADD,
                    )
                    nc.scalar.activation(out=out_ap, in_=z, func=Tanh)

                # Wave 0: GEMM block 0 per group, sweep-1A chases, then the
                # carry slot flips from c1 (seed of sweep 1) to c2 (seed of
                # sweep 2).
                for g in range(NG):
                    gemm(0, g)
                    sweep(g, 0, U[g][:, 1:1 + TB], f"1A{g}")
                    nc.vector.tensor_copy(
                        out=U[g][:, 0:1], in_=cr_sb[:, g, 1:2]
                    )
                # Wave 1: GEMM block 1, sweep-1B, then both sweep-2 blocks
                # (sweep-2A only needed sweep-1A; emitted here to keep each
                # engine queue in dependency-ready order) and the output DMA.
                for g in range(NG):
                    gemm(1, g)
                    sweep(g, TB, U[g][:, 1 + TB:1 + Tc], f"1B{g}")
                    O = opool.tile([128, Tc], F16, tag="O", name=f"O{g}")
                    sweep(g, 0, O[:, 0:TB], f"2A{g}")
                    sweep(g, TB, O[:, TB:Tc], f"2B{g}")
                    nc.scalar.dma_start(out=ht_r[:, g, :], in_=O)

            if loop_iters == 1:
                body(0)
            else:
                with tc.For_i(
                    0, loop_iters, 1, hint_engines=(mybir.EngineType.PE,)
                ) as i:
                    body(i)

    nc.compile()
    return nc


def _build_runner(nc):
    """Reusable jitted shard_map executable for an 8-core SPMD Bass module."""
    install_neuronx_cc_hook()
    partition_name = nc.partition_id_tensor.name if nc.partition_id_tensor else None
    in_names, out_names, out_avals = [], [], []
    for alloc in nc.m.functions[0].allocations:
        if not isinstance(alloc, mybir.MemoryLocationSet):
            continue
        name = alloc.memorylocations[0].name
        if alloc.kind == "ExternalInput":
            if name != partition_name:
                in_names.append(name)
        elif alloc.kind == "ExternalOutput":
            out_names.append(name)
            out_avals.append(
                jax.core.ShapedArray(
                    tuple(alloc.tensor_shape), mybir.dt.np(alloc.dtype)
                )
            )
    n_params = len(in_names)
    n_outs = len(out_avals)
    in_names_all = list(in_names) + list(out_names)
    if partition_name is not None:
        in_names_all.append(partition_name)
    donate = tuple(range(n_params, n_params + n_outs))

    def _bdy(*args):
        operands = list(args)
        if partition_name is not None:
            operands.append(partition_id_tensor())
        return tuple(
            _bass_exec_p.bind(
                *operands,
                out_avals=tuple(out_avals),
                in_names=tuple(in_names_all),
                out_names=tuple(out_names),
                lowering_input_output_aliases=(),
                sim_require_finite=True,
                sim_require_nnan=True,
                nc=nc,
            )
        )

    devices = jax.devices()[:N_CORES]
    mesh = Mesh(np.asarray(devices), ("core",))
    in_specs = (PartitionSpec("core"),) * (n_params + n_outs)
    out_specs = (PartitionSpec("core"),) * len(out_names)
    sharded = jax.jit(
        shard_map(
            _bdy, mesh=mesh, in_specs=in_specs, out_specs=out_specs,
            check_rep=False,
        ),
        donate_argnums=donate,
        keep_unused=True,
    )
    shardng = NamedSharding(mesh, PartitionSpec("core"))
    out_shapes = [
        (N_CORES * a.shape[0], *a.shape[1:]) for a in out_avals
    ]
    out_dtypes = [a.dtype for a in out_avals]

    class Runner:
        def put_inputs(self, in_maps):
            concat = [
                np.concatenate([m[n] for m in in_maps], axis=0) for n in in_names
            ]
            return [jax.device_put(a, shardng) for a in concat]

        def zeros(self):
            return [
                jax.device_put(np.zeros(s, d), shardng)
                for s, d in zip(out_shapes, out_dtypes)
            ]

        def exec_device(self, dev_in, dev_zeros):
            outs = sharded(*dev_in, *dev_zeros)
            jax.block_until_ready(outs)
            return outs

        def fetch(self, outs):
            return {
                name: np.asarray(outs[i]).reshape(N_CORES, -1, *out_avals[i].shape[1:])
                for i, name in enumerate(out_names)
            }

        def __call__(self, dev_in, dev_zeros):
            return self.fetch(self.exec_device(dev_in, dev_zeros))

    return Runner()


_CACHE: dict = {}


def get_compiled(loop_iters=1):
    key = loop_iters
    if key not in _CACHE:
        nc = _build(loop_iters)
        _CACHE[key] = (nc, _build_runner(nc))
    return _CACHE[key]


def make_in_maps(x, a_mat, b_mat):
    x = np.ascontiguousarray(np.asarray(x, np.float32))
    a_mat = np.ascontiguousarray(np.asarray(a_mat, np.float32))
    b_mat = np.ascontiguousarray(np.asarray(b_mat, np.float32))
    xT16 = np.ascontiguousarray(x.T).astype(np.float16)   # [K, T]
    b16 = b_mat.astype(np.float16)
    # host carries: fp32 s rows at each chunk boundary (t0-2, t0-1)
    bnd_rows = []
    for i in range(1, P_T):
        bnd_rows += [i * Tc - 2, i * Tc - 1]
    s_bnd = x[bnd_rows] @ b_mat                            # [2*(P_T-1), H] fp32
    in_maps = []
    for c in range(N_CORES):
        i, j = divmod(c, Q_H)
        h0 = j * Hc
        av32 = a_mat[h0:h0 + Hc]
        if i == 0:
            c1 = np.zeros(Hc, np.float32)
            c2 = np.zeros(Hc, np.float32)
        else:
            sA = s_bnd[2 * (i - 1), h0:h0 + Hc]
            sB = s_bnd[2 * (i - 1) + 1, h0:h0 + Hc]
            c1 = np.tanh(sB)
            c2 = np.tanh(av32 * np.tanh(sA) + sB)
        cr = np.stack([c1, c2], -1).astype(np.float16)     # [Hc, 2]
        in_maps.append(
            {
                "xt": np.ascontiguousarray(xT16[:, i * Tc:(i + 1) * Tc]),
                "bt": np.ascontiguousarray(b16[:, h0:h0 + Hc]),
                "cr": np.ascontiguousarray(
                    cr.reshape(NG, 128, 2).transpose(1, 0, 2)
                ),
                "av": np.ascontiguousarray(
                    av32.astype(np.float16).reshape(NG, 128).T
                ),
            }
        )
    return in_maps


def kernel(x, a_mat, b_mat):
    from concourse import bass_utils

    key = "nc1"
    if key not in _CACHE:
        _CACHE[key] = _build(1)
    nc = _CACHE[key]
    in_maps = make_in_maps(x, a_mat, b_mat)
    res = bass_utils.run_bass_kernel_spmd(nc, in_maps, core_ids=list(range(N_CORES)))
    out = np.empty((T, H), np.float32)
    for c in range(N_CORES):
        i, j = divmod(c, Q_H)
        ht = np.asarray(res.results[c]["ht"])              # [Hc, Tc] fp16
        out[i * Tc:(i + 1) * Tc, j * Hc:(j + 1) * Hc] = ht.T.astype(np.float32)
    return out


# revision 7
# speedup vs baseline: 1.2965x; 1.2965x over previous
"""Trainium2 Bass kernel for nn_DiagSSMBlock: h_t = tanh(a * h_{t-1} + (x @ b)_t).

Strategy (8 NeuronCores, 2D shard: 4 T-chunks x 2 H-halves => PE-bound):
  - The per-core GEMM work is fixed (34.4 GFLOP / 8 = 4.3 GFLOP, ~54.6us at
    the fp16 PE rate of 1 row/cycle @ 2.4 GHz), so the only lever is HBM
    traffic. The old H-only sharding broadcast the full 16MB fp16 x to every
    core (~61us DMA > PE). The 4x2 grid moves per core only:
      x slice  [K=2048, Tc=1024] fp16 = 4.2MB
      b half   [K=2048, Hc=1024] fp16 = 4.2MB
      out      [Hc, Tc]          fp16 = 2.1MB
    => ~10.5MB ~ 29us, fully hidden under the PE stream.
  - The diagonal recurrence is per-channel independent; T-sharding is made
    embarrassingly parallel by the same Gauss-Seidel fixed-point trick as
    before (|a| <= 0.03125, tanh 1-Lipschitz => each sweep contracts error
    by |a|). Chunk carries are resolved on the HOST: for each chunk start
    t0, two fp32 GEMV rows s_{t0-2}, s_{t0-1} (50 MFLOP total) give
      c1 = u0[t0-1] = tanh(s_{t0-1})
      c2 = u1[t0-1] = tanh(a*tanh(s_{t0-2}) + s_{t0-1})
    which seed sweep 1 / sweep 2 via a carry slot at U[:, 0]. Scan error
    after u0 + 2 sweeps is ~|a|^3 ~ 3e-5; the fp16 GEMM rounding (~9e-4 of
    output scale) dominates. Measured end-to-end rel err ~1.0e-3 (tol 2e-2).
  - All elementwise state (s, u, z, out) is fp16: 2x DVE rate and half the
    output DMA. Output is cast back to fp32 on host.
"""

import numpy as np

import jax
from jax.sharding import Mesh, NamedSharding, PartitionSpec
from jax.experimental.shard_map import shard_map

import concourse.tile as tile
from concourse import bacc, mybir
from concourse.bass2jax import (
    _bass_exec_p,
    install_neuronx_cc_hook,
    partition_id_tensor,
)

T = 4096          # sequence length
K = 2048          # input features (contraction dim)
H = 2048          # output channels
N_CORES = 8
P_T = 4           # t-chunks
Q_H = 2           # h-halves
Tc = T // P_T     # 1024 time steps per core
Hc = H // Q_H     # 1024 channels per core
NG = Hc // 128    # 8 channel groups of 128 partitions
KT = K // 128     # 16 k-tiles
TB = 512          # GEMM moving-dim block (one PSUM bank of fp32)
NB = Tc // TB     # 2 t-blocks

F32 = mybir.dt.float32
F16 = mybir.dt.float16


def _build(loop_iters: int, unroll: int = 1):
    nc = bacc.Bacc(
        "TRN2", target_bir_lowering=False, debug=False, num_devices=N_CORES
    )

    xt_d = nc.dram_tensor("xt", [K, Tc], F16, kind="ExternalInput").ap()
    bt_d = nc.dram_tensor("bt", [K, Hc], F16, kind="ExternalInput").ap()
    cr_d = nc.dram_tensor("cr", [128, NG, 2], F16, kind="ExternalInput").ap()
    av_d = nc.dram_tensor("av", [128, NG], F16, kind="ExternalInput").ap()
    ht_d = nc.dram_tensor("ht", [Hc, Tc], F16, kind="ExternalOutput").ap()

    xt_r = xt_d.rearrange("(kt p) t -> p kt t", p=128)
    bt_r = bt_d.rearrange("(kt p) c -> p kt c", p=128)
    ht_r = ht_d.rearrange("(g p) t -> p g t", g=NG)

    Tanh = mybir.ActivationFunctionType.Tanh
    MUL = mybir.AluOpType.mult
    ADD = mybir.AluOpType.add

    with tile.TileContext(nc) as tc:
        with (
            tc.tile_pool(name="state", bufs=1) as state,
            tc.tile_pool(name="xp", bufs=2) as xpool,
            tc.tile_pool(name="bp", bufs=2) as bpool,
            tc.tile_pool(name="ps", bufs=3, space="PSUM") as psum,
            tc.tile_pool(name="zp", bufs=4) as zpool,
            tc.tile_pool(name="op", bufs=3) as opool,
        ):
            # constants: loaded once, live across all iterations
            a_sb = state.tile([128, NG], F16, tag="a")
            cr_sb = state.tile([128, NG, 2], F16, tag="cr")
            nc.sync.dma_start(out=a_sb, in_=av_d)
            nc.sync.dma_start(out=cr_sb, in_=cr_d)

            def body(_i, sfx=""):
                # x: one rotating tile per body (full slice); b: 4 rotating
                # quad-group tiles. Rotation (pool bufs >= allocations per
                # loop emission x 2 bodies) lets iteration n+1's input DMAs
                # run during iteration n (no WAR stall on the PE stream).
                x_sb = xpool.tile([128, KT, Tc], F16, tag="x", name=f"x{sfx}")
                b_sb = [
                    bpool.tile([128, KT, 256], F16, tag=f"b{h}", name=f"b{h}{sfx}")
                    for h in range(4)
                ]
                U = [
                    state.tile([128, Tc + 1], F16, tag=f"U{g}", name=f"U{g}{sfx}")
                    for g in range(NG)
                ]
                sT = [
                    state.tile([128, Tc], F16, tag=f"sT{g}", name=f"sT{g}{sfx}")
                    for g in range(NG)
                ]

                nc.sync.dma_start(out=b_sb[0], in_=bt_r[:, :, 0:256])
                for c in range(4):
                    nc.sync.dma_start(
                        out=x_sb[:, :, c * 256:(c + 1) * 256],
                        in_=xt_r[:, :, c * 256:(c + 1) * 256],
                    )
                for h in range(1, 4):
                    nc.sync.dma_start(
                        out=b_sb[h], in_=bt_r[:, :, h * 256:(h + 1) * 256]
                    )
                for g in range(NG):
                    nc.vector.tensor_copy(
                        out=U[g][:, 0:1], in_=cr_sb[:, g, 0:1]
                    )

                def gemm(g):
                    for tb in range(NB):
                        ps = psum.tile([128, TB], F32, tag="ps")
                        lo = tb * TB
                        for kt in range(KT):
                            nc.tensor.matmul(
                                ps,
                                lhsT=b_sb[g // 2][:, kt, (g % 2) * 128:(g % 2 + 1) * 128],
                                rhs=x_sb[:, kt, lo:lo + TB],
                                start=(kt == 0),
                                stop=(kt == KT - 1),
                            )
                        nc.vector.tensor_copy(out=sT[g][:, lo:lo + TB], in_=ps)
                        nc.scalar.activation(
                            out=U[g][:, 1 + lo:1 + lo + TB], in_=ps, func=Tanh
                        )

                def sweep(g, lo, out_ap, tag):
                    z = zpool.tile([128, TB], F16, tag="z", name=f"z_{tag}{sfx}")
                    nc.vector.scalar_tensor_tensor(
                        out=z,
                        in0=U[g][:, lo:lo + TB],
                        scalar=a_sb[:, g:g + 1],
                        in1=sT[g][:, lo:lo + TB],
                        op0=MUL,
                        op1=ADD,
                    )
                    nc.scalar.activation(out=out_ap, in_=z, func=Tanh)

                for g in range(NG):
                    gemm(g)
                    sweep(g, 0, U[g][:, 1:1 + TB], f"1A{g}")
                    nc.vector.tensor_copy(
                        out=U[g][:, 0:1], in_=cr_sb[:, g, 1:2]
                    )
                    sweep(g, TB, U[g][:, 1 + TB:1 + Tc], f"1B{g}")
                    O = opool.tile([128, Tc], F16, tag="O", name=f"O{g}{sfx}")
                    sweep(g, 0, O[:, 0:TB], f"2A{g}")
                    sweep(g, TB, O[:, TB:Tc], f"2B{g}")
                    nc.scalar.dma_start(out=ht_r[:, g, :], in_=O)

            if loop_iters == 1:
                for u in range(unroll):
                    body(u, sfx=f"_u{u}" if unroll > 1 else "")
            else:
                # two bodies per hardware-loop trip (even/odd pool buffers)
                # so input prefetch crosses the back edge; trip count is
                # halved to keep total work = loop_iters bodies (rounded up
                # to even, consistently, so timing-by-difference is exact).
                trips = (loop_iters + 1) // 2
                with tc.For_i(
                    0, trips, 1, hint_engines=(mybir.EngineType.PE,)
                ) as i:
                    body(i, "_a")
                    body(i, "_b")

    nc.compile()
    return nc


def _build_runner(nc):
    """Reusable jitted shard_map executable for an 8-core SPMD Bass module."""
    install_neuronx_cc_hook()
    partition_name = nc.partition_id_tensor.name if nc.partition_id_tensor else None
    in_names, out_names, out_avals = [], [], []
    for alloc in nc.m.functions[0].allocations:
        if not isinstance(alloc, mybir.MemoryLocationSet):
            continue
        name = alloc.memorylocations[0].name
        if alloc.kind == "ExternalInput":
            if name != partition_name:
                in_names.append(name)
        elif alloc.kind == "ExternalOutput":
            out_names.append(name)
            out_avals.append(
                jax.core.ShapedArray(
                    tuple(alloc.tensor_shape), mybir.dt.np(alloc.dtype)
                )
            )
    n_params = len(in_names)
    n_outs = len(out_avals)
    in_names_all = list(in_names) + list(out_names)
    if partition_name is not None:
        in_names_all.append(partition_name)
    donate = tuple(range(n_params, n_params + n_outs))

    def _bdy(*args):
        operands = list(args)
        if partition_name is not None:
            operands.append(partition_id_tensor())
        return tuple(
            _bass_exec_p.bind(
                *operands,
                out_avals=tuple(out_avals),
                in_names=tuple(in_names_all),
                out_names=tuple(out_names),
                lowering_input_output_aliases=(),
                sim_require_finite=True,
                sim_require_nnan=True,
                nc=nc,
            )
        )

    devices = jax.devices()[:N_CORES]
    mesh = Mesh(np.asarray(devices), ("core",))
    in_specs = (PartitionSpec("core"),) * (n_params + n_outs)
    out_specs = (PartitionSpec("core"),) * len(out_names)
    sharded = jax.jit(
        shard_map(
            _bdy, mesh=mesh, in_specs=in_specs, out_specs=out_specs,
            check_rep=False,
        ),
        donate_argnums=donate,
        keep_unused=True,
    )
    shardng = NamedSharding(mesh, PartitionSpec("core"))
    out_shapes = [
        (N_CORES * a.shape[0], *a.shape[1:]) for a in out_avals
    ]
    out_dtypes = [a.dtype for a in out_avals]

    class Runner:
        def put_inputs(self, in_maps):
            concat = [
                np.concatenate([m[n] for m in in_maps], axis=0) for n in in_names
            ]
            return [jax.device_put(a, shardng) for a in concat]

        def zeros(self):
            return [
                jax.device_put(np.zeros(s, d), shardng)
                for s, d in zip(out_shapes, out_dtypes)
            ]

        def exec_device(self, dev_in, dev_zeros):
            outs = sharded(*dev_in, *dev_zeros)
            jax.block_until_ready(outs)
            return outs

        def fetch(self, outs):
            return {
                name: np.asarray(outs[i]).reshape(N_CORES, -1, *out_avals[i].shape[1:])
                for i, name in enumerate(out_names)
            }

        def __call__(self, dev_in, dev_zeros):
            return self.fetch(self.exec_device(dev_in, dev_zeros))

    return Runner()


_CACHE: dict = {}


def get_compiled(loop_iters=1):
    key = loop_iters
    if key not in _CACHE:
        nc = _build(loop_iters)
        _CACHE[key] = (nc, _build_runner(nc))
    return _CACHE[key]


def make_in_maps(x, a_mat, b_mat):
    x = np.ascontiguousarray(np.asarray(x, np.float32))
    a_mat = np.ascontiguousarray(np.asarray(a_mat, np.float32))
    b_mat = np.ascontiguousarray(np.asarray(b_mat, np.float32))
    xT16 = np.ascontiguousarray(x.T).astype(np.float16)   # [K, T]
    b16 = b_mat.astype(np.float16)
    # host carries: fp32 s rows at each chunk boundary (t0-2, t0-1)
    bnd_rows = []
    for i in range(1, P_T):
        bnd_rows += [i * Tc - 2, i * Tc - 1]
    s_bnd = x[bnd_rows] @ b_mat                            # [2*(P_T-1), H] fp32
    in_maps = []
    for c in range(N_CORES):
        i, j = divmod(c, Q_H)
        h0 = j * Hc
        av32 = a_mat[h0:h0 + Hc]
        if i == 0:
            c1 = np.zeros(Hc, np.float32)
            c2 = np.zeros(Hc, np.float32)
        else:
            sA = s_bnd[2 * (i - 1), h0:h0 + Hc]
            sB = s_bnd[2 * (i - 1) + 1, h0:h0 + Hc]
            c1 = np.tanh(sB)
            c2 = np.tanh(av32 * np.tanh(sA) + sB)
        cr = np.stack([c1, c2], -1).astype(np.float16)     # [Hc, 2]
        in_maps.append(
            {
                "xt": np.ascontiguousarray(xT16[:, i * Tc:(i + 1) * Tc]),
                "bt": np.ascontiguousarray(b16[:, h0:h0 + Hc]),
                "cr": np.ascontiguousarray(
                    cr.reshape(NG, 128, 2).transpose(1, 0, 2)
                ),
                "av": np.ascontiguousarray(
                    av32.astype(np.float16).reshape(NG, 128).T
                ),
            }
        )
    return in_maps


def kernel(x, a_mat, b_mat):
    from concourse import bass_utils

    key = "nc1"
    if key not in _CACHE:
        _CACHE[key] = _build(1)
    nc = _CACHE[key]
    in_maps = make_in_maps(x, a_mat, b_mat)
    res = bass_utils.run_bass_kernel_spmd(nc, in_maps, core_ids=list(range(N_CORES)))
    out = np.empty((T, H), np.float32)
    for c in range(N_CORES):
        i, j = divmod(c, Q_H)
        ht = np.asarray(res.results[c]["ht"])              # [Hc, Tc] fp16
        out[i * Tc:(i + 1) * Tc, j * Hc:(j + 1) * Hc] = ht.T.astype(np.float32)
    return out
